# revision 9
# baseline (speedup 1.0000x reference)
"""DotGatConv (DGL dot-product graph attention) Trainium2 kernel, 8 NeuronCores.

Device strategy (edge-parallel over dst, feature-space projection on PE):
  * Nodes are degree-sorted and dealt into 8*NB blocks of 128 dst nodes
    (superblock j gives one 128-node block per core, padded degree K_j,
    k-major slot grid: slot = k*128 + r).
  * The host marshals inputs: featsrcT[:, slot] = feat[src(slot)].T (bf16,
    zero for pad slots) and featdstT[:, j*128+r] = feat[node(j,r)].T.
    The projection h = feat @ W runs on-device (PE, cached W weights):
      hsrcT = W^T @ featsrcT   (per k-slab)
      hdstT = W^T @ featdstT   (per block)
  * Scores: DVE bf16 product against k-broadcast hdstT, PE head-selector
    matmul reduces features, ACT exp (scale 1/sqrt(32), no max-subtraction
    needed: |e| < 4).
  * Softmax denominator: GPSIMD tensor_reduce over k; pad correction adds
    -(K-deg) (exp(0)=1 per pad slot).
  * Aggregation: PE identity-matmul PSUM accumulation over k-slices of
    exT * hsrcT.
  * Output stays feature-major [128, r]; host inverse-permutes/transposes.
"""

import sys

sys.path.insert(0, "/opt/trn_rl_repo")

import numpy as np
import ml_dtypes

import concourse.bacc as bacc
import concourse.mybir as mybir
import concourse.tile as tile
from concourse.bass_utils import run_bass_kernel_spmd
from concourse.masks import make_identity

BF16 = mybir.dt.bfloat16
F32 = mybir.dt.float32

NCORES = 8
P = 128
F = 128  # H * D
H = 4
D = 32
SCALE = 1.0 / np.sqrt(32.0)
SLAB = 32  # k-columns per compute slab (32*128 = 4096 slots)


def _prep(src, dst, N, NB):
    """Host-side schedule: degree-sorted fixed-K blocks, k-major slot grid."""
    NPAD = NB * 1024
    E = len(src)
    src = np.asarray(src, np.int64).ravel()
    dst = np.asarray(dst, np.int64).ravel()
    if np.any(np.diff(dst) < 0):
        p = np.argsort(dst, kind="stable")
        src, dst = src[p], dst[p]

    deg = np.bincount(dst, minlength=NPAD).astype(np.int64)
    order = np.argsort(-deg, kind="stable")
    rank = np.empty(NPAD, np.int64)
    rank[order] = np.arange(NPAD)
    degs_sorted = deg[order]
    Kj = np.maximum(degs_sorted.reshape(NB, 1024).max(1), 2).astype(np.int64)
    # round K up to a multiple of SLAB? No: slabs may be ragged; just even.
    Kj = Kj + (Kj & 1)

    slot_off_j = np.zeros(NB + 1, np.int64)
    np.cumsum(128 * Kj, out=slot_off_j[1:])
    TOT = int(slot_off_j[-1])

    starts = np.zeros(NPAD + 1, np.int64)
    np.cumsum(deg, out=starts[1:])
    k_e = np.arange(E, dtype=np.int64) - starts[dst]
    rho = rank[dst]
    j_e, q = rho // 1024, rho % 1024
    c_e, r_e = q // 128, q % 128

    # per-core slot -> src map (-1 = pad)
    src_of_slot = np.full((NCORES, TOT), -1, np.int64)
    slots = slot_off_j[j_e] + k_e * 128 + r_e
    src_of_slot[c_e, slots] = src

    ids_all = order.reshape(NB, NCORES, 128)  # [j, c, r]
    npadneg = np.zeros((NCORES, NB * 128), np.float32)
    for c in range(NCORES):
        for j in range(NB):
            ids_b = ids_all[j, c, :]
            corr = -(Kj[j] - deg[ids_b]).astype(np.float32)
            corr[deg[ids_b] == 0] = -(Kj[j] - 1)
            npadneg[c, j * 128 : (j + 1) * 128] = corr

    return dict(
        N=N, E=E, NB=NB, NPAD=NPAD, Kj=Kj, TOT=TOT,
        slot_off_j=slot_off_j, order=order, deg=deg,
        src_of_slot=src_of_slot, ids_all=ids_all,
        npadneg=npadneg,
    )


def _build(sched):
    NB = sched["NB"]
    Kj = sched["Kj"]
    TOT = sched["TOT"]
    slot_off_j = sched["slot_off_j"]

    nc = bacc.Bacc("TRN2")
    t_fs = nc.dram_tensor("fsrcT", [P, TOT], BF16, kind="ExternalInput")
    t_fd = nc.dram_tensor("fdstT", [P, NB * 128], BF16, kind="ExternalInput")
    t_np = nc.dram_tensor("npadneg", [P, NB * 128], BF16, kind="ExternalInput")
    t_W = nc.dram_tensor("W", [F, F], F32, kind="ExternalInput")
    t_out = nc.dram_tensor("outT", [P, NB * 128], F32, kind="ExternalOutput")

    with tile.TileContext(nc) as tc:
        with (
            tc.tile_pool(name="cst", bufs=1) as cst,
            tc.tile_pool(name="fs", bufs=3) as fsp,
            tc.tile_pool(name="hs", bufs=3) as hsp,
            tc.tile_pool(name="wrk", bufs=2) as wrk,
            tc.tile_pool(name="blk", bufs=2) as blk,
            tc.tile_pool(name="sml", bufs=2) as sml,
            tc.tile_pool(name="psP", bufs=2, space="PSUM") as psP,
            tc.tile_pool(name="psE", bufs=2, space="PSUM") as psE,
            tc.tile_pool(name="psA", bufs=2, space="PSUM") as psA,
        ):
            wf = cst.tile([F, F], F32)
            wb = cst.tile([F, F], BF16)
            nc.sync.dma_start(out=wf[:, :], in_=t_W[:, :])
            nc.vector.tensor_copy(out=wb[:, :], in_=wf[:, :])
            ident = cst.tile([P, P], BF16)
            make_identity(nc, ident[:])
            headsel = cst.tile([P, P], BF16)
            nc.vector.memset(headsel[:, :], 0.0)
            for hh in range(H):
                nc.vector.memset(
                    headsel[hh * D : (hh + 1) * D, hh * D : (hh + 1) * D], 1.0
                )
            npn = cst.tile([P, NB * 128], BF16)
            nc.sync.dma_start(out=npn[:, :], in_=t_np[:, :])

            for j in range(NB):
                K = int(Kj[j])
                eb = 128 * K
                base = int(slot_off_j[j])

                # hdstT for this block: W^T @ fdT
                fd = sml.tile([P, 128], BF16, tag="fd")
                nc.sync.dma_start(
                    out=fd[:, :], in_=t_fd[:, j * 128 : (j + 1) * 128]
                )
                hd_ps = psP.tile([P, 512], F32, tag="hp")
                nc.tensor.matmul(
                    out=hd_ps[:, :128], lhsT=wb[:, :], rhs=fd[:, :],
                    start=True, stop=True,
                )
                hd = sml.tile([P, 128], BF16, tag="hd_sb")
                nc.vector.tensor_copy(out=hd[:, :], in_=hd_ps[:, :128])

                exT = blk.tile([P, eb], BF16, tag="exT")
                agg = psA.tile([P, 128], F32, tag="agg")
                s_ps = psA.tile([P, 128], F32, tag="s")

                nslab = (K + SLAB - 1) // SLAB
                for si in range(nslab):
                    k0 = si * SLAB
                    kw = min(SLAB, K - k0)
                    sw = kw * 128
                    sbase = base + k0 * 128
                    fs = fsp.tile([P, SLAB * 128], BF16, tag="fs")
                    nc.sync.dma_start(
                        out=fs[:, :sw], in_=t_fs[:, sbase : sbase + sw]
                    )
                    # project: hsrcT = W^T @ fs  (pieces of 512)
                    hs = hsp.tile([P, SLAB * 128], BF16, tag="hs")
                    for p0 in range(0, sw, 512):
                        pw = min(512, sw - p0)
                        h_ps = psP.tile([P, 512], F32, tag="hp")
                        nc.tensor.matmul(
                            out=h_ps[:, :pw], lhsT=wb[:, :],
                            rhs=fs[:, p0 : p0 + pw], start=True, stop=True,
                        )
                        # PSUM -> SBUF bf16 (split between ACT and DVE)
                        if (p0 // 512) % 2 == 0:
                            nc.scalar.activation(
                                out=hs[:, p0 : p0 + pw], in_=h_ps[:, :pw],
                                func=mybir.ActivationFunctionType.Copy,
                            )
                        else:
                            nc.vector.tensor_copy(
                                out=hs[:, p0 : p0 + pw], in_=h_ps[:, :pw]
                            )
                    # prod = hs * bcast_k(hd)
                    prod = wrk.tile([P, SLAB * 128], BF16, tag="prod")
                    nc.vector.tensor_tensor(
                        out=prod[:, :sw].rearrange("p (k r) -> p k r", r=128),
                        in0=hs[:, :sw].rearrange("p (k r) -> p k r", r=128),
                        in1=hd[:, :].unsqueeze(1).to_broadcast([P, kw, 128]),
                        op=mybir.AluOpType.mult,
                    )
                    # e_rep pieces + exp
                    for p0 in range(0, sw, 512):
                        pw = min(512, sw - p0)
                        e_ps = psE.tile([P, 512], F32, tag="e")
                        nc.tensor.matmul(
                            out=e_ps[:, :pw], lhsT=headsel[:, :],
                            rhs=prod[:, p0 : p0 + pw], start=True, stop=True,
                        )
                        nc.scalar.activation(
                            out=exT[:, k0 * 128 + p0 : k0 * 128 + p0 + pw],
                            in_=e_ps[:, :pw],
                            func=mybir.ActivationFunctionType.Exp,
                            scale=float(SCALE),
                        )
                    # exhs = exT * hs ; aggregate over k into PSUM
                    exhs = wrk.tile([P, SLAB * 128], BF16, tag="exhs")
                    nc.vector.tensor_tensor(
                        out=exhs[:, :sw],
                        in0=exT[:, k0 * 128 : k0 * 128 + sw],
                        in1=hs[:, :sw],
                        op=mybir.AluOpType.mult,
                    )
                    for k in range(kw):
                        nc.tensor.matmul(
                            out=agg[:, :], lhsT=ident[:, :],
                            rhs=exhs[:, k * 128 : (k + 1) * 128],
                            start=(si == 0 and k == 0),
                            stop=(si == nslab - 1 and k == kw - 1),
                        )
                    for k in range(kw):
                        kk = k0 + k
                        nc.tensor.matmul(
                            out=s_ps[:, :], lhsT=ident[:, :],
                            rhs=exT[:, kk * 128 : (kk + 1) * 128],
                            start=(si == 0 and k == 0),
                            stop=(si == nslab - 1 and k == kw - 1),
                        )

                # s += npadneg (pad slots each contributed exp(0)=1)
                s_c = sml.tile([P, 128], F32, tag="sc")
                nc.vector.tensor_tensor(
                    out=s_c[:, :], in0=s_ps[:, :],
                    in1=npn[:, j * 128 : (j + 1) * 128],
                    op=mybir.AluOpType.add,
                )
                rs = sml.tile([P, 128], F32, tag="rs")
                nc.vector.reciprocal(rs[:, :], s_c[:, :])
                outF = sml.tile([P, 128], F32, tag="outF")
                nc.vector.tensor_tensor(
                    out=outF[:, :], in0=agg[:, :], in1=rs[:, :],
                    op=mybir.AluOpType.mult,
                )
                nc.sync.dma_start(
                    out=t_out[:, j * 128 : (j + 1) * 128], in_=outF[:, :]
                )
    nc.compile()
    return nc


def _marshal(feat, W, sched):
    """Per-core host input marshaling (index-driven replication only)."""
    N, NPAD, NB = sched["N"], sched["NPAD"], sched["NB"]
    featT = np.zeros((P, NPAD), np.float32)
    featT[:, :N] = np.asarray(feat, np.float32).T
    featT_bf = featT.astype(ml_dtypes.bfloat16)
    zero_col = np.zeros((P,), ml_dtypes.bfloat16)

    in_maps = []
    for c in range(NCORES):
        sos = sched["src_of_slot"][c]
        fs = featT_bf[:, np.where(sos >= 0, sos, 0)]
        fs[:, sos < 0] = 0
        ids = sched["ids_all"][:, c, :].reshape(-1)  # [NB*128]
        fd = featT_bf[:, ids]
        npn = np.broadcast_to(
            sched["npadneg"][c][None, :], (P, NB * 128)
        ).astype(ml_dtypes.bfloat16)
        in_maps.append(
            {
                "fsrcT": np.ascontiguousarray(fs),
                "fdstT": np.ascontiguousarray(fd),
                "npadneg": np.ascontiguousarray(npn),
                "W": np.asarray(W, np.float32),
            }
        )
    return in_maps


def _assemble(res, sched):
    NB, NPAD, N = sched["NB"], sched["NPAD"], sched["N"]
    allT = np.stack([np.asarray(res[c]["outT"]) for c in range(NCORES)])
    X = allT.reshape(NCORES, F, NB, 128).transpose(2, 0, 3, 1)  # [j, c, r, f]
    out_full = np.empty((NPAD, F), np.float32)
    out_full[sched["order"].reshape(NB, NCORES, 128)] = X
    out = out_full[:N].reshape(N, H, D).astype(np.float32)
    isolated = sched["deg"][:N] == 0
    if isolated.any():
        out[isolated] = 0.0
    return out


def _run(feat, src, dst, W, N, NB):
    sched = _prep(src, dst, N, NB)
    nc = _build(sched)
    in_maps = _marshal(feat, W, sched)
    res = run_bass_kernel_spmd(nc, in_maps, core_ids=list(range(NCORES)))
    out = _assemble(res.results, sched)
    _run.last = dict(nc=nc, in_maps=in_maps, sched=sched)
    return out, res


def kernel(feat, src, dst, W):
    out, _ = _run(feat, src, dst, W, N=50000, NB=49)
    return out
